# revision 18
# baseline (speedup 1.0000x reference)
"""CLIP captioning model (dense transformer decoder) on 8 trn2 NeuronCores.

Sharding: data-parallel over batch B=16 -> 2 items per core; weights
replicated. Each core runs the full 6-layer decoder + LM head over the
whole vocab for its 2 sequences. No collectives.

Matmuls run as float32r (full PE rate at free dim >= 256, ~1e-4 rel err).
LayerNorm gamma/beta are folded into the following weight matrices on the
host (exact algebra); bias terms are emitted on device only when nonzero.
"""

import os

import ml_dtypes
import numpy as np

import concourse.bacc as bacc
import concourse.bass as bass
import concourse.mybir as mybir
import concourse.tile as tile
from concourse.bass import IndirectOffsetOnAxis
from concourse.bass_utils import run_bass_kernel_spmd
from concourse.masks import make_identity

P = 128
B, T, S = 16, 128, 50
E, H, HS, L, V, CD = 512, 8, 64, 6, 32000, 768
EPS = 1e-5
NCORES = 8
BPC = B // NCORES          # batch items per core
NT = BPC                   # token tiles per core (T == P)
ESUB = E // P              # 4
CSUB = CD // P             # 6
MH = 4 * E                 # 2048
MSUB = MH // P             # 16
VCHUNK = 512
HB = E // P                # head-blocks (2 heads of 64 per 128-partition block)

F32 = mybir.dt.float32
F32R = mybir.dt.float32r
I32 = mybir.dt.int32
AF = mybir.ActivationFunctionType
ALU = mybir.AluOpType
AX = mybir.AxisListType

NEG_BIG = -3.0e38


def _prep_host(params):
    """Fold LN affine params into weights; reshape per-head weights."""
    p = {k: np.asarray(v, dtype=np.float32) if np.asarray(v).dtype != np.int64 else np.asarray(v)
         for k, v in params.items()}

    def headcat(w):  # [L,H,E,HS] -> [L,E,H*HS]
        return np.ascontiguousarray(w.transpose(0, 2, 1, 3).reshape(L, E, H * HS))

    out = {}
    out["temb"] = np.ascontiguousarray(p["tok_emb"])                    # [V,E]
    out["pos"] = np.ascontiguousarray(p["pos_emb"])                     # [T,E]
    out["wvp"] = np.ascontiguousarray(p["Wvp"])                         # [CD,E]
    out["bvp"] = np.ascontiguousarray(p["bvp"])                         # [E]

    scale = HS ** -0.5
    g1 = p["ln1_g"][:, :, None]  # [L,E,1]
    b1 = p["ln1_b"]              # [L,E]
    wq = headcat(p["Wq1"]) * scale
    wk = headcat(p["Wk1"])
    wv = headcat(p["Wv1"])
    out["wqa"] = np.ascontiguousarray(g1 * wq)
    out["wka"] = np.ascontiguousarray(g1 * wk)
    out["wva"] = np.ascontiguousarray(g1 * wv)
    out["bqa"] = np.einsum("le,lek->lk", b1, wq)
    out["bka"] = np.einsum("le,lek->lk", b1, wk)
    out["bva"] = np.einsum("le,lek->lk", b1, wv)
    out["woa"] = np.ascontiguousarray(p["Wo1"])                         # [L,E,E]
    out["boa"] = np.ascontiguousarray(p["bo1"])                         # [L,E]

    g2 = p["ln2_g"][:, :, None]
    b2 = p["ln2_b"]
    wqx = headcat(p["Wq2"]) * scale
    out["wqx"] = np.ascontiguousarray(g2 * wqx)
    out["bqx"] = np.einsum("le,lek->lk", b2, wqx)
    out["wkx"] = headcat(p["Wk2"])
    out["wvx"] = headcat(p["Wv2"])
    out["wox"] = np.ascontiguousarray(p["Wo2"])
    out["box"] = np.ascontiguousarray(p["bo2"])

    g3 = p["ln3_g"][:, :, None]
    b3 = p["ln3_b"]
    out["w1"] = np.ascontiguousarray(g3 * p["W1"])                      # [L,E,MH]
    out["b1f"] = p["b1"] + np.einsum("le,lem->lm", b3, out["w1"])       # [L,MH]
    out["w2"] = np.ascontiguousarray(p["W2"])                           # [L,MH,E]
    out["b2f"] = np.ascontiguousarray(p["b2"])                          # [L,E]

    gf = p["lnf_g"][:, None]
    bf = p["lnf_b"]
    out["wlm"] = np.ascontiguousarray(gf * p["W_lm"])                   # [E,V]
    out["blm"] = p["b_lm"] + bf @ out["wlm"]                            # [V]

    out["maskend"] = np.arange(1, T + 1, dtype=np.float32)              # [T]
    ma = np.zeros((T, T), dtype=np.float32)
    ma[np.arange(T)[:, None] < np.arange(T)[None, :]] = -1.0e30
    out["maskadd"] = ma
    return out


def _nz(a):
    return bool(np.any(a != 0))


def _build_program(flags, layers=L, stages=("self", "cross", "mlp"), lm_head=True, dump_x=False,
                   WDT=F32R, HDT=F32R):
    """flags: bias nonzero map. WDT: matmul dtype for layers; HDT: lm head."""
    nc = bacc.Bacc("TRN2", target_bir_lowering=False, debug=False,
                   num_devices=NCORES)

    def din(name, shape, dt):
        return nc.declare_dram_parameter(name, list(shape), dt, isOutput=False)

    # per-core inputs
    tokens = din("tokens_i32", [NT * T, 1], I32)
    imgT = din("imgT", [CD, BPC * S], WDT)
    # shared weights
    temb = din("temb", [V, E], F32)
    pos = din("pos", [T, E], F32)
    wvp = din("wvp", [CD, E], WDT)
    bvp = din("bvp", [E, 1], F32) if flags["bvp"] else None
    wqa = din("wqa", [L, E, E], WDT)
    wka = din("wka", [L, E, E], WDT)
    wva = din("wva", [L, E, E], WDT)
    bqa = din("bqa", [L, E, 1], F32) if flags["bqa"] else None
    bka = din("bka", [L, E, 1], F32) if flags["bka"] else None
    bva = din("bva", [L, E], WDT) if flags["bva"] else None
    woa = din("woa", [L, E, E], WDT)
    boa = din("boa", [L, E], WDT) if flags["boa"] else None
    wqx = din("wqx", [L, E, E], WDT)
    bqx = din("bqx", [L, E, 1], F32) if flags["bqx"] else None
    wkx = din("wkx", [L, E, E], WDT)
    wvx = din("wvx", [L, E, E], WDT)
    wox = din("wox", [L, E, E], WDT)
    box = din("box", [L, E], WDT) if flags["box"] else None
    w1 = din("w1", [L, E, MH], WDT)
    b1f = din("b1f", [L, MH], F32) if flags["b1f"] else None
    w2 = din("w2", [L, MH, E], WDT)
    b2f = din("b2f", [L, E], WDT) if flags["b2f"] else None
    wlm = din("wlm", [E, V], HDT)
    blm = din("blm", [V], HDT) if flags["blm"] else None
    maskend = din("maskend", [T, 1], F32)
    maskadd = din("maskadd", [T, T], F32)

    logits = nc.declare_dram_parameter("logits", [NT * T, V], F32, isOutput=True)

    def w_rearr(ap):  # [K, N] dram -> [P, K//P, N]
        return ap.rearrange("(s p) c -> p s c", p=P)

    with tile.TileContext(nc) as tc:
        with (
            tc.tile_pool(name="singles", bufs=1) as singles,
            tc.tile_pool(name="acts", bufs=1) as acts,
            tc.tile_pool(name="small", bufs=4) as small,
            tc.tile_pool(name="wpool", bufs=4) as wpool,
            tc.tile_pool(name="wmlp_pool", bufs=2) as wmlp_pool,
            tc.tile_pool(name="evict", bufs=4) as evict,
            tc.tile_pool(name="psA", bufs=2, space="PSUM") as psA,
            tc.tile_pool(name="psB", bufs=2, space="PSUM") as psB,
            tc.tile_pool(name="psS", bufs=3, space="PSUM") as psS,
        ):
            # ---- constants ----
            ident = singles.tile([P, P], F32)
            make_identity(nc, ident)
            eps_t = singles.tile([P, 1], F32)
            nc.vector.memset(eps_t, EPS)
            mend = singles.tile([P, 1], F32)
            nc.sync.dma_start(out=mend, in_=maskend[:])
            madd4 = singles.tile([P, 4, T], F32)
            nc.sync.dma_start(out=madd4,
                              in_=maskadd[:, None, :].to_broadcast([T, 4, T]))
            pos_t = singles.tile([P, E], F32)
            nc.sync.dma_start(out=pos_t, in_=pos[:])
            ones_r = None
            need_ones = any(flags[k] for k in ("bva", "boa", "box", "b2f", "blm"))
            if need_ones:
                ones_r = singles.tile([P, P], WDT)
                nc.vector.memset(ones_r, 1.0)

            # ---- persistent activations ----
            x_sb = singles.tile([P, NT, E], F32)          # residual stream
            encT = singles.tile([P, ESUB, BPC * S], WDT)  # enc transposed

            # ---- embeddings ----
            with nc.named_scope("embed"):
                for tt in range(NT):
                    idx = small.tile([P, 1], I32, tag="idx")
                    nc.sync.dma_start(out=idx, in_=tokens[tt * T:(tt + 1) * T, :])
                    xg = acts.tile([P, E], F32, tag="xg")
                    nc.gpsimd.indirect_dma_start(
                        out=xg[:], out_offset=None, in_=temb[:],
                        in_offset=IndirectOffsetOnAxis(ap=idx[:, :1], axis=0),
                    )
                    nc.vector.tensor_add(out=x_sb[:, tt, :], in0=xg[:], in1=pos_t[:])

            # ---- visual projection: encT[e, s] directly ----
            with nc.named_scope("visproj"):
                imgT_sb = acts.tile([P, CSUB, BPC * S], WDT, tag="imgT")
                nc.sync.dma_start(out=imgT_sb, in_=w_rearr(imgT[:]))
                wvp_sb = acts.tile([P, CSUB, E], WDT, tag="wvp")
                nc.sync.dma_start(out=wvp_sb, in_=w_rearr(wvp[:]))
                bvp_sb = None
                if bvp is not None:
                    bvp_sb = acts.tile([P, ESUB], F32, tag="bvp")
                    nc.sync.dma_start(out=bvp_sb,
                                      in_=bvp[:].rearrange("(s p) o -> p (s o)", p=P))
                for eb in range(ESUB):
                    ps = psB.tile([P, BPC * S], F32, tag="ps_big")
                    for cs in range(CSUB):
                        nc.tensor.matmul(ps, lhsT=wvp_sb[:, cs, eb * P:(eb + 1) * P],
                                         rhs=imgT_sb[:, cs, :],
                                         start=(cs == 0), stop=(cs == CSUB - 1))
                    if bvp_sb is not None:
                        nc.scalar.activation(out=encT[:, eb, :], in_=ps,
                                             func=AF.Identity,
                                             bias=bvp_sb[:, eb:eb + 1])
                    else:
                        nc.any.tensor_copy(out=encT[:, eb, :], in_=ps)

            def layernorm(src, dst, tag):
                """src/dst [P, E] APs; dst <- (src - mu) * rsqrt(var + eps)."""
                st = small.tile([P, 6], F32, tag=f"st_{tag}")
                nc.vector.bn_stats(out=st, in_=src)
                mv = small.tile([P, 2], F32, tag=f"mv_{tag}")
                nc.vector.bn_aggr(out=mv, in_=st)
                nc.scalar.activation(out=mv[:, 1:2], in_=mv[:, 1:2],
                                     func=AF.Sqrt, bias=eps_t[:, :1])
                nc.vector.reciprocal(out=mv[:, 1:2], in_=mv[:, 1:2])
                nc.vector.tensor_scalar(
                    out=dst, in0=src, scalar1=mv[:, 0:1], scalar2=mv[:, 1:2],
                    op0=ALU.subtract, op1=ALU.mult)

            def transpose_to(hT, h):
                """h [P, NT, E] f32 -> hT [P, ESUB, NT*T] f32r via PE."""
                for tt in range(NT):
                    for es in range(ESUB):
                        pt = psS.tile([P, P], F32, tag="ps_sm")
                        nc.tensor.transpose(pt, h[:, tt, es * P:(es + 1) * P], ident)
                        nc.any.tensor_copy(out=hT[:, es, tt * T:(tt + 1) * T],
                                           in_=pt)

            def proj_T(dst, w_sb, hT, bias_sb, n_free, tagps):
                """dst[p, hb, n] (f32r) <- (W^T h^T)[hk, n] + bias per hk."""
                for hb in range(HB):
                    ps = psA.tile([P, n_free], F32, tag=tagps)
                    for es in range(ESUB):
                        nc.tensor.matmul(ps, lhsT=w_sb[:, es, hb * P:(hb + 1) * P],
                                         rhs=hT[:, es, :n_free],
                                         start=(es == 0), stop=(es == ESUB - 1))
                    if bias_sb is not None:
                        nc.scalar.activation(out=dst[:, hb, :n_free], in_=ps,
                                             func=AF.Identity,
                                             bias=bias_sb[:, hb:hb + 1])
                    else:
                        nc.any.tensor_copy(out=dst[:, hb, :n_free], in_=ps)

            def load_w4(dram_ap, tag="w4"):
                wt = wpool.tile([P, ESUB, E], WDT, tag=tag)
                nc.sync.dma_start(out=wt, in_=w_rearr(dram_ap))
                return wt

            def load_bias_col(dram_ap, tag):
                # [E,1] dram -> [P, ESUB] sbuf (per-partition bias per block)
                bt = small.tile([P, ESUB], F32, tag=tag)
                nc.sync.dma_start(out=bt,
                                  in_=dram_ap.rearrange("(s p) o -> p (s o)", p=P))
                return bt

            def bias_row_matmul(ps_ap, dram_row_ap, width, dt=WDT):
                # ps += ones^T @ bias_row ; K=1 matmul broadcast over partitions
                br = small.tile([P, VCHUNK], dt, tag="brow")
                nc.sync.dma_start(out=br[:1, :width], in_=dram_row_ap[None, :])
                nc.tensor.matmul(ps_ap, lhsT=ones_r[:1, :ps_ap.shape[0]],
                                 rhs=br[:1, :width], start=False, stop=True)

            # =================== decoder layers ===================
            for l in range(layers):
                # ---------- self-attention ----------
                if "self" not in stages:
                    pass
                elif True:
                  with nc.named_scope(f"L{l}_selfattn"):
                    h = acts.tile([P, NT, E], F32, tag="h")
                    for tt in range(NT):
                        layernorm(x_sb[:, tt, :], h[:, tt, :], "ln")
                    hT = acts.tile([P, ESUB, NT * T], WDT, tag="hT")
                    transpose_to(hT, h)

                    wq_sb = load_w4(wqa[l])
                    bq_sb = load_bias_col(bqa[l], "bq") if bqa is not None else None
                    QT = acts.tile([P, HB, NT * T], WDT, tag="QT")
                    proj_T(QT, wq_sb, hT, bq_sb, NT * T, "ps_a")

                    wk_sb = load_w4(wka[l])
                    bk_sb = load_bias_col(bka[l], "bk") if bka is not None else None
                    KT = acts.tile([P, HB, NT * T], WDT, tag="KT")
                    proj_T(KT, wk_sb, hT, bk_sb, NT * T, "ps_a")

                    wv_sb = load_w4(wva[l])
                    Vt = acts.tile([P, NT, E], WDT, tag="Vt")
                    for tt in range(NT):
                        ps = psB.tile([P, E], F32, tag="ps_big")
                        for es in range(ESUB):
                            nc.tensor.matmul(ps, lhsT=hT[:, es, tt * T:(tt + 1) * T],
                                             rhs=wv_sb[:, es, :],
                                             start=(es == 0),
                                             stop=(es == ESUB - 1 and bva is None))
                        if bva is not None:
                            bias_row_matmul(ps, bva[l], E)
                        nc.any.tensor_copy(out=Vt[:, tt, :], in_=ps)

                    attn = acts.tile([P, NT, E], F32, tag="attn")
                    for tt in range(NT):
                        pa = psA.tile([P, E], F32, tag="ps_a")
                        for hd in range(H):
                            pr = (hd % 2) * HS
                            ps_s = psS.tile([P, T], F32, tag="ps_sm")
                            nc.tensor.matmul(
                                ps_s,
                                lhsT=QT[pr:pr + HS, hd // 2, tt * T:(tt + 1) * T],
                                rhs=KT[pr:pr + HS, hd // 2, tt * T:(tt + 1) * T],
                                start=True, stop=True)
                            wei = small.tile([P, T], F32, tag="wei")
                            nmax = small.tile([P, 1], F32, tag="nmax")
                            nc.vector.tensor_add(out=wei, in0=ps_s, in1=madd4[:, 0, :])
                            nc.vector.reduce_max(out=nmax, in_=wei,
                                                 axis=AX.X, negate=True)
                            ssum = small.tile([P, 1], F32, tag="ssum")
                            nc.scalar.activation(out=wei, in_=wei, func=AF.Exp,
                                                 bias=nmax[:, :1],
                                                 accum_out=ssum[:, :1])
                            rinv = small.tile([P, 1], F32, tag="rinv")
                            nc.vector.reciprocal(out=rinv, in_=ssum)
                            nc.vector.tensor_scalar_mul(out=wei, in0=wei,
                                                        scalar1=rinv[:, :1])
                            pt = psS.tile([P, T], F32, tag="ps_sm")
                            nc.tensor.transpose(pt, wei, ident)
                            wt = small.tile([P, T], WDT, tag="wt")
                            nc.any.tensor_copy(out=wt, in_=pt)
                            nc.tensor.matmul(
                                out=pa[:, hd * HS:(hd + 1) * HS],
                                lhsT=wt,
                                rhs=Vt[:, tt, hd * HS:(hd + 1) * HS],
                                start=True, stop=True)
                        nc.any.tensor_copy(out=attn[:, tt, :], in_=pa)
                    oT = acts.tile([P, HB, NT * T], WDT, tag="oT")
                    transpose_to(oT, attn)

                    wo_sb = load_w4(woa[l])
                    for tt in range(NT):
                        ps = psB.tile([P, E], F32, tag="ps_big")
                        for kb in range(ESUB):
                            nc.tensor.matmul(ps, lhsT=oT[:, kb, tt * T:(tt + 1) * T],
                                             rhs=wo_sb[:, kb, :],
                                             start=(kb == 0),
                                             stop=(kb == ESUB - 1 and boa is None))
                        if boa is not None:
                            bias_row_matmul(ps, boa[l], E)
                        nc.vector.tensor_add(out=x_sb[:, tt, :],
                                             in0=x_sb[:, tt, :], in1=ps)

                # ---------- cross-attention ----------
                if "cross" not in stages:
                    pass
                elif True:
                  with nc.named_scope(f"L{l}_crossattn"):
                    h = acts.tile([P, NT, E], F32, tag="h")
                    for tt in range(NT):
                        layernorm(x_sb[:, tt, :], h[:, tt, :], "ln")
                    hT = acts.tile([P, ESUB, NT * T], WDT, tag="hT")
                    transpose_to(hT, h)

                    wq_sb = load_w4(wqx[l])
                    bqx_sb = load_bias_col(bqx[l], "bq") if bqx is not None else None
                    QT = acts.tile([P, HB, NT * T], WDT, tag="QT")
                    proj_T(QT, wq_sb, hT, bqx_sb, NT * T, "ps_a")

                    wk_sb = load_w4(wkx[l])
                    KXT = acts.tile([P, HB, BPC * S], WDT, tag="KXT")
                    proj_T(KXT, wk_sb, encT, None, BPC * S, "ps_a")

                    wv_sb = load_w4(wvx[l])
                    VX = acts.tile([P, NT, E], WDT, tag="VX")
                    for tt in range(BPC):
                        ps = psB.tile([P, E], F32, tag="ps_big")
                        for es in range(ESUB):
                            nc.tensor.matmul(
                                ps[:S, :],
                                lhsT=encT[:, es, tt * S:(tt + 1) * S],
                                rhs=wv_sb[:, es, :],
                                start=(es == 0), stop=(es == ESUB - 1))
                        nc.any.tensor_copy(out=VX[:S, tt, :], in_=ps[:S, :])

                    attn = acts.tile([P, NT, E], F32, tag="attn")
                    for tt in range(NT):
                        pa = psA.tile([P, E], F32, tag="ps_a")
                        for hd in range(H):
                            pr = (hd % 2) * HS
                            ps_s = psS.tile([P, T], F32, tag="ps_sm")
                            nc.tensor.matmul(
                                ps_s[:, :S],
                                lhsT=QT[pr:pr + HS, hd // 2, tt * T:(tt + 1) * T],
                                rhs=KXT[pr:pr + HS, hd // 2, tt * S:(tt + 1) * S],
                                start=True, stop=True)
                            wei = small.tile([P, T], F32, tag="wei")
                            nmax = small.tile([P, 1], F32, tag="nmax")
                            nc.vector.reduce_max(out=nmax, in_=ps_s[:, :S],
                                                 axis=AX.X, negate=True)
                            ssum = small.tile([P, 1], F32, tag="ssum")
                            nc.scalar.activation(out=wei[:, :S], in_=ps_s[:, :S],
                                                 func=AF.Exp, bias=nmax[:, :1],
                                                 accum_out=ssum[:, :1])
                            rinv = small.tile([P, 1], F32, tag="rinv")
                            nc.vector.reciprocal(out=rinv, in_=ssum)
                            nc.vector.tensor_scalar_mul(out=wei[:, :S],
                                                        in0=wei[:, :S],
                                                        scalar1=rinv[:, :1])
                            pt = psS.tile([P, T], F32, tag="ps_sm")
                            nc.tensor.transpose(pt[:S, :], wei[:, :S], ident)
                            wt = small.tile([P, T], WDT, tag="wt")
                            nc.any.tensor_copy(out=wt[:S, :], in_=pt[:S, :])
                            nc.tensor.matmul(
                                out=pa[:, hd * HS:(hd + 1) * HS],
                                lhsT=wt[:S, :],
                                rhs=VX[:S, tt, hd * HS:(hd + 1) * HS],
                                start=True, stop=True)
                        nc.any.tensor_copy(out=attn[:, tt, :], in_=pa)
                    oT = acts.tile([P, HB, NT * T], WDT, tag="oT")
                    transpose_to(oT, attn)

                    wo_sb = load_w4(wox[l])
                    for tt in range(NT):
                        ps = psB.tile([P, E], F32, tag="ps_big")
                        for kb in range(ESUB):
                            nc.tensor.matmul(ps, lhsT=oT[:, kb, tt * T:(tt + 1) * T],
                                             rhs=wo_sb[:, kb, :],
                                             start=(kb == 0),
                                             stop=(kb == ESUB - 1 and box is None))
                        if box is not None:
                            bias_row_matmul(ps, box[l], E)
                        nc.vector.tensor_add(out=x_sb[:, tt, :],
                                             in0=x_sb[:, tt, :], in1=ps)

                # ---------- MLP ----------
                if "mlp" not in stages:
                    pass
                elif True:
                  with nc.named_scope(f"L{l}_mlp"):
                    h = acts.tile([P, NT, E], F32, tag="h")
                    for tt in range(NT):
                        layernorm(x_sb[:, tt, :], h[:, tt, :], "ln")
                    hT = acts.tile([P, ESUB, NT * T], WDT, tag="hT")
                    transpose_to(hT, h)

                    b1_sb = None
                    if b1f is not None:
                        b1_sb = small.tile([P, MSUB], F32, tag="b1")
                        nc.sync.dma_start(
                            out=b1_sb,
                            in_=b1f[l].rearrange("(s p) -> p s", p=P))
                    h1T = acts.tile([P, MSUB, NT * T], WDT, tag="h1T")
                    for half in range(2):
                        w1t = wmlp_pool.tile([P, ESUB, MH // 2], WDT, tag="wmlp")
                        nc.sync.dma_start(
                            out=w1t,
                            in_=w_rearr(w1[l])[:, :, half * (MH // 2):(half + 1) * (MH // 2)])
                        for mi in range(MSUB // 2):
                            mb = half * (MSUB // 2) + mi
                            ps = psA.tile([P, NT * T], F32, tag="ps_a")
                            for es in range(ESUB):
                                nc.tensor.matmul(
                                    ps, lhsT=w1t[:, es, mi * P:(mi + 1) * P],
                                    rhs=hT[:, es, :],
                                    start=(es == 0), stop=(es == ESUB - 1))
                            nc.scalar.activation(
                                out=h1T[:, mb, :], in_=ps, func=AF.Gelu,
                                bias=(b1_sb[:, mb:mb + 1] if b1_sb is not None else 0.0))

                    w2a = wmlp_pool.tile([P, MSUB // 2, E], WDT, tag="wmlp")
                    nc.sync.dma_start(out=w2a, in_=w_rearr(w2[l])[:, :MSUB // 2, :])
                    w2b = wmlp_pool.tile([P, MSUB // 2, E], WDT, tag="wmlp")
                    nc.sync.dma_start(out=w2b, in_=w_rearr(w2[l])[:, MSUB // 2:, :])
                    for tt in range(NT):
                        ps = psB.tile([P, E], F32, tag="ps_big")
                        for mb in range(MSUB):
                            w2t = w2a if mb < MSUB // 2 else w2b
                            nc.tensor.matmul(
                                ps, lhsT=h1T[:, mb, tt * T:(tt + 1) * T],
                                rhs=w2t[:, mb % (MSUB // 2), :],
                                start=(mb == 0),
                                stop=(mb == MSUB - 1 and b2f is None))
                        if b2f is not None:
                            bias_row_matmul(ps, b2f[l], E)
                        nc.vector.tensor_add(out=x_sb[:, tt, :],
                                             in0=x_sb[:, tt, :], in1=ps)

            # =================== LM head ===================
            if dump_x:
                for tt in range(NT):
                    nc.sync.dma_start(out=logits[tt * T:(tt + 1) * T, :E],
                                      in_=x_sb[:, tt, :])
            if lm_head:
              with nc.named_scope("lmhead"):
                xf = acts.tile([P, NT, E], F32, tag="h")
                for tt in range(NT):
                    layernorm(x_sb[:, tt, :], xf[:, tt, :], "ln")
                xfT = acts.tile([P, ESUB, NT * T], HDT, tag="hT")
                transpose_to(xfT, xf)

                wlm_r = w_rearr(wlm[:])  # [P, ESUB, V]
                for c0 in range(0, V, VCHUNK):
                    cw = min(VCHUNK, V - c0)
                    wl = wpool.tile([P, ESUB, VCHUNK], HDT, tag="w4")
                    nc.sync.dma_start(out=wl[:, :, :cw],
                                      in_=wlm_r[:, :, c0:c0 + cw])
                    for tt in range(NT):
                        ps = psB.tile([P, VCHUNK], F32, tag="ps_big")
                        for es in range(ESUB):
                            nc.tensor.matmul(
                                ps[:, :cw], lhsT=xfT[:, es, tt * T:(tt + 1) * T],
                                rhs=wl[:, es, :cw],
                                start=(es == 0),
                                stop=(es == ESUB - 1 and blm is None))
                        if blm is not None:
                            bias_row_matmul(ps[:, :cw], blm[c0:c0 + cw], cw, dt=HDT)
                        lo = evict.tile([P, VCHUNK], F32, tag="lsb")
                        nc.any.tensor_copy(out=lo[:, :cw], in_=ps[:, :cw])
                        nc.sync.dma_start(
                            out=logits[tt * T:(tt + 1) * T, c0:c0 + cw],
                            in_=lo[:, :cw])

    nc.compile()
    return nc


_PROGRAM_CACHE = {}


def kernel(tokens, img_features, params):
    tokens = np.asarray(tokens)
    img_features = np.asarray(img_features, dtype=np.float32)
    hp = _prep_host(params)

    flags = {
        "bvp": _nz(hp["bvp"]), "bqa": _nz(hp["bqa"]), "bka": _nz(hp["bka"]),
        "bva": _nz(hp["bva"]), "boa": _nz(hp["boa"]), "bqx": _nz(hp["bqx"]),
        "box": _nz(hp["box"]), "b1f": _nz(hp["b1f"]), "b2f": _nz(hp["b2f"]),
        "blm": _nz(hp["blm"]),
    }
    mode = os.environ.get("BASSK_MODE", "f32r")
    WDT = F32R if mode == "f32r" else mybir.dt.bfloat16
    HDT = F32R if mode in ("f32r",) else mybir.dt.bfloat16
    if mode == "head_bf16":
        WDT = F32R
    key = (mode, tuple(sorted(flags.items())))
    if key not in _PROGRAM_CACHE:
        _PROGRAM_CACHE[key] = _build_program(flags, WDT=WDT, HDT=HDT)
    nc = _PROGRAM_CACHE[key]
    wnp = np.float32 if WDT == F32R else ml_dtypes.bfloat16
    hnp = np.float32 if HDT == F32R else ml_dtypes.bfloat16

    shared = {
        "temb": hp["temb"], "pos": hp["pos"],
        "wvp": hp["wvp"].astype(wnp),
        "wqa": hp["wqa"].astype(wnp), "wka": hp["wka"].astype(wnp),
        "wva": hp["wva"].astype(wnp), "woa": hp["woa"].astype(wnp),
        "wqx": hp["wqx"].astype(wnp), "wkx": hp["wkx"].astype(wnp),
        "wvx": hp["wvx"].astype(wnp), "wox": hp["wox"].astype(wnp),
        "w1": hp["w1"].astype(wnp), "w2": hp["w2"].astype(wnp),
        "wlm": hp["wlm"].astype(hnp), "maskend": hp["maskend"][:, None],
        "maskadd": hp["maskadd"],
    }
    if flags["bvp"]:
        shared["bvp"] = hp["bvp"][:, None]
    if flags["bqa"]:
        shared["bqa"] = hp["bqa"][:, :, None]
    if flags["bka"]:
        shared["bka"] = hp["bka"][:, :, None]
    if flags["bva"]:
        shared["bva"] = hp["bva"]
    if flags["boa"]:
        shared["boa"] = hp["boa"]
    if flags["bqx"]:
        shared["bqx"] = hp["bqx"][:, :, None]
    if flags["box"]:
        shared["box"] = hp["box"]
    if flags["b1f"]:
        shared["b1f"] = hp["b1f"]
    if flags["b2f"]:
        shared["b2f"] = hp["b2f"]
    if flags["blm"]:
        shared["blm"] = hp["blm"]

    in_maps = []
    for c in range(NCORES):
        toks = tokens[c * BPC:(c + 1) * BPC].reshape(-1, 1).astype(np.int32)
        img = img_features[c * BPC:(c + 1) * BPC]          # [BPC, S, CD]
        imgT = np.ascontiguousarray(
            img.reshape(BPC * S, CD).T).astype(wnp)        # [CD, BPC*S]
        in_maps.append({"tokens_i32": toks, "imgT": imgT, **shared})

    res = run_bass_kernel_spmd(nc, in_maps, core_ids=list(range(NCORES)))
    out = np.empty((B, T, V), dtype=np.float32)
    for c in range(NCORES):
        out[c * BPC:(c + 1) * BPC] = res.results[c]["logits"].reshape(BPC, T, V)
    return out
